# revision 30
# baseline (speedup 1.0000x reference)
"""KalmanNet SLAM DNN forward pass on a single Trainium2 NeuronCore.

Network: x(14) -> Linear(560)+ReLU -> GRUCell(145) -> GRUCell(145)
         -> Linear(40)+ReLU -> Linear(10) -> reshape (5,2)

~1.8MB of fp32 weights, single sample => memory-bound; replicate on one
core (per sharding hint).

v4: weights/activations in fp16 (halves HBM traffic; PE runs 16-bit
matmuls at 1 cycle/row at any pstate, vs fp32r's 4x penalty), psum fp32.
mc goes out on the Sync HWDGE ring (desc-gen concurrent with GpSimd's);
the weight images stream through the gpsimd SWDGE queue in compute
order (wih0-half1 -> whh0+tails -> wih0-half2 -> whh1 -> rest), each
with its own completion semaphore so consumers unblock as their slice
lands.  GRU1's h-dependent matmuls (whh1 @ h1) are emitted before
GRU0's pointwise chain so the PE keeps working through the
Scalar/Vector hops; rz matmuls get scheduler priority so the sigmoid
fires as early as possible, and the gate math is arranged as
h' = (1-z)*n + z*h with u=1-z and z*h computed while tanh runs.

Matvecs run weights-stationary on the TensorEngine.  Activation vectors
kept in duplicated column pairs ([K,2] rhs -> [M,2] psum) end to end.

Host-side numpy packs everything into partition-major DRAM images,
weights pre-transposed to [K, M] layout, biases folded as an extra
weight row against a constant-1.0 input element, GRU gates padded
145->146 so output chunks are uniform 73 partitions, and the z-gate
pad-column bias set to 100 so the h' garbage slot computes to exactly
the 1.0 the next bias row needs.
"""

import numpy as np

import concourse.bacc as bacc
import concourse.mybir as mybir
import concourse.tile as tile
from concourse import bass_utils
from concourse.alu_op_type import AluOpType as ALU

F32 = mybir.dt.float32
F16 = mybir.dt.float16
AF = mybir.ActivationFunctionType

X_DIM, Y_DIM = 5, 2
H1, H2 = 560, 40
G = 145          # GRU hidden size
C = 73           # partition chunk for the GRU state (2*73 = 146 = G+1)
GP = 2 * C       # per-gate padded column block
M3 = 3 * GP      # 438 padded gate columns
NO = X_DIM * Y_DIM

RZ = 2 * GP      # 292 rz columns per chunk (gru0 split layout)
NB = GP          # 146 n columns per chunk

# megaB (128-partition image) column map
B_H0, B_H1 = 0, 4                      # h ptwise pairs [73,4]: (c0,c0,c1,c1)
B_HK = 8                               # whh rhs chunks: h0[128], h1[128],
                                       # tails (h0 rows 64:82, h1 rows 96:114)
B_WHH0RZ = 14                          # [128, 292] whh0 rz cols
B_TRZ = B_WHH0RZ + RZ                  # [*, 292] tails rz: rows 0:49 wih0c4,
                                       #   64:82 whh0t, 96:114 whh1t
B_WHH0N = B_TRZ + RZ                   # [128, 146] whh0 n cols
B_TN = B_WHH0N + NB                    # [*, 146] tails n cols
B_MB1 = B_TN + NB                      # 890: end of DMA chunk 1 (rz part ends B_WHH0N)
B_WHH1 = B_MB1                         # [128, 438] K-chunk 0 (DMA chunk 2)
B_MB2 = B_WHH1 + M3                    # 1328: end of DMA chunk 2
B_WIH1 = B_MB2                         # 2 x 438 (73-row chunks, DMA chunk 3)
B_W2A = B_WIH1 + 2 * M3                # 2 x 41 (41st col makes the 1.0)
B_W2B = B_W2A + 2 * (H2 + 1)           # [41,10]
B_F = B_W2B + NO                       # 2296

A_F = 4 * M3                           # mega128: wih0 rz c0..c3 then n c0..c3
A_RZ = 4 * RZ                          # 1168: end of wih0 rz block
C_F = H1 + 3                           # megaC: W1T(561, unit col) + x pair
# whh tails live in the mc image at rows 0:18 (PE-row subset of the mains)
C_W0TRZ = C_F                          # [18, 292] whh0 tail rz
C_W0TN = C_W0TRZ + RZ                  # [18, 146] whh0 tail n
C_W1TRZ = C_W0TN + NB                  # [18, 292] whh1 tail rz
C_W1TN = C_W1TRZ + RZ                  # [18, 146] whh1 tail n
C_H0T = C_W1TN + NB                    # [18, 2] h0 tail rhs pair
C_H1T = C_H0T + 2                      # [18, 2] h1 tail rhs pair
C_F2 = C_H1T + 2                       # 1447 total mc cols

TRACE = False
_BUILT = None


def _tp(chunk):
    return chunk[2] if len(chunk) > 2 else None


def _gru_mm_rz(nc, ps_rz, chunks, i0, n_total):
    """Emit rz matmuls for `chunks` into ps_rz (accumulation i0..)."""
    i = i0
    for ch in chunks:
        lhsT, rhs = ch[0], ch[1]
        for g in (0, 1):
            for c in (0, 1):
                j = 2 * g + c
                nc.tensor.matmul(
                    ps_rz[:, 2 * j: 2 * j + 2],
                    lhsT[:, g * GP + C * c: g * GP + C * (c + 1)],
                    rhs, start=(i == 0), stop=(i == n_total - 1),
                    tile_position=_tp(ch), skip_group_check=True)
                i += 1
    return i


def _gru_mm_n(nc, ps, chunks, ni0, nn_total):
    """Emit n-gate matmuls for `chunks` into ps.  Each chunk is
    (rz_lhsT, rhs, tile_position|None, n_lhsT); the n block is sliced from
    the chunk's own n_lhsT base."""
    n_i = ni0
    for ch in chunks:
        n_lhsT, rhs = ch[3], ch[1]
        for c in (0, 1):
            nc.tensor.matmul(
                ps[:, 2 * c: 2 * c + 2],
                n_lhsT[:, C * c: C * (c + 1)],
                rhs, start=(n_i == 0), stop=(n_i == nn_total - 1),
                tile_position=_tp(ch), skip_group_check=True)
            n_i += 1
    return n_i


def _gru_pt(nc, ab, name, ps_rz, ps_ni, ps_nh, h_sb):
    """r,z = sigmoid(rz sums); n = tanh(i_n + r*h_n);
    h' = (1-z)*n + z*h.  u=1-z and zh=z*h are computed while tanh runs so
    only two dependent Vector ops remain after it.
    Returns h' [73,4] F16 pairs."""
    rz = ab.tile([C, 8], F32, tag=f"{name}_rz")
    nc.scalar.activation(rz[:, 0:4], ps_rz[:, 0:4], AF.Sigmoid)
    nc.scalar.activation(rz[:, 4:8], ps_rz[:, 4:8], AF.Sigmoid)
    t1 = ab.tile([C, 4], F32, tag=f"{name}_t1")
    nc.vector.tensor_mul(t1, rz[:, 0:4], ps_nh)
    nc.vector.tensor_add(t1, t1, ps_ni)
    n_sb = ab.tile([C, 4], F32, tag=f"{name}_n")
    nc.scalar.activation(n_sb, t1, AF.Tanh)
    u = ab.tile([C, 4], F32, tag=f"{name}_u")
    nc.vector.tensor_scalar(u, rz[:, 4:8], -1.0, 1.0, ALU.mult, ALU.add)
    zh = ab.tile([C, 4], F32, tag=f"{name}_zh")
    nc.vector.tensor_mul(zh, rz[:, 4:8], h_sb)
    t2 = ab.tile([C, 4], F32, tag=f"{name}_t2")
    nc.vector.tensor_mul(t2, n_sb, u)
    hp = ab.tile([C, 4], F16, tag=f"{name}_hp")
    nc.vector.tensor_add(hp, t2, zh)
    return hp


def _build():
    nc = bacc.Bacc("TRN2", num_devices=1, num_swdge_queues=4)

    d_c = nc.dram_tensor("mega_c", [18, C_F2], F16, kind="ExternalInput").ap()
    d_a1 = nc.dram_tensor("mega_a1", [128, A_RZ], F16, kind="ExternalInput").ap()
    d_a2 = nc.dram_tensor("mega_a2", [128, A_F - A_RZ], F16, kind="ExternalInput").ap()
    d_b1 = nc.dram_tensor("mega_b1", [128, B_WHH0N], F16, kind="ExternalInput").ap()
    d_b1n = nc.dram_tensor("mega_b1n", [128, B_MB1 - B_WHH0N], F16,
                           kind="ExternalInput").ap()
    d_b23 = nc.dram_tensor("mega_b23", [128, B_F - B_MB1], F16,
                           kind="ExternalInput").ap()
    d_out = nc.dram_tensor("out", [1, NO], F32, kind="ExternalOutput").ap()

    with tile.TileContext(nc) as tc:
        with (
            tc.tile_pool(name="wp", bufs=1) as wp,
            tc.tile_pool(name="ab", bufs=1) as ab,
            tc.tile_pool(name="pp", bufs=1, space="PSUM") as pp,
        ):
            # --- DMAs first: mc on the Sync HWDGE ring (concurrent
            # desc-gen), weights on SWDGE q0 in compute order ---
            mc = wp.tile([18, C_F2], F16, tag="mc")
            nc.sync.dma_start(mc, d_c)
            ma = wp.tile([128, A_F], F16, tag="ma")
            mb = wp.tile([128, B_F], F16, tag="mb")
            nc.gpsimd.dma_start(ma[:, 0:A_RZ], d_a1)
            nc.gpsimd.dma_start(mb[:, 0:B_WHH0N], d_b1)
            nc.gpsimd.dma_start(ma[:, A_RZ:A_F], d_a2)
            nc.gpsimd.dma_start(mb[:, B_WHH0N:B_MB1], d_b1n)
            nc.gpsimd.dma_start(mb[:, B_MB1:B_F], d_b23)

            # ACT table warmup (pulls the table-set load to t~0 on Scalar)
            warm = ab.tile([1, 1], F32, tag="warm")
            nc.vector.memset(warm, 0.0)
            warm2 = ab.tile([1, 1], F32, tag="warm2")
            nc.scalar.activation(warm2, warm, AF.Sigmoid)
            nc.scalar.activation(warm2, warm2, AF.Tanh)

            # h pairs to fp32 for the pointwise math (Vector idle then)
            h0_sb = ab.tile([C, 4], F32, tag="h0c")
            nc.vector.tensor_copy(h0_sb, mb[0:C, B_H0:B_H0 + 4])
            h1_sb = ab.tile([C, 4], F32, tag="h1c")
            nc.vector.tensor_copy(h1_sb, mb[0:C, B_H1:B_H1 + 4])

            # --- layer 1: l1 = relu(W1 @ x + b1), [128,10] paired cols;
            # relu split per chunk so gru0 chunk c can start asap ---
            x2 = mc[0:15, H1 + 1:H1 + 3]
            l1_sb = ab.tile([128, 10], F16, tag="l1")
            ps_l1 = pp.tile([128, 8], F32, tag="p0")
            for c in range(4):
                nc.tensor.matmul(ps_l1[:, 2 * c:2 * c + 2],
                                 mc[0:15, c * 128:(c + 1) * 128], x2,
                                 start=(c == 0), stop=(c == 3),
                                 skip_group_check=True)
            ps_l1b = pp.tile([49, 2], F32, tag="p5")
            nc.tensor.matmul(ps_l1b, mc[0:15, 512:561], x2,
                             start=True, stop=True)
            nc.vector.tensor_scalar_max(l1_sb[:, 0:8], ps_l1, 0.0)
            nc.vector.tensor_scalar_max(l1_sb[0:49, 8:10], ps_l1b, 0.0)

            # --- GRU 0 matmuls (split rz/n image layout) ---
            wih0_chunks = [
                (ma[:, c * RZ:(c + 1) * RZ], l1_sb[:, 2 * c:2 * c + 2], None,
                 ma[:, A_RZ + c * NB:A_RZ + (c + 1) * NB])
                for c in range(4)
            ] + [
                (mb[0:49, B_TRZ:B_TRZ + RZ], l1_sb[0:49, 8:10], None,
                 mb[0:49, B_TN:B_TN + NB])
            ]
            whh0_chunks = [
                (mb[0:128, B_WHH0RZ:B_WHH0RZ + RZ], mb[0:128, B_HK:B_HK + 2],
                 None, mb[0:128, B_WHH0N:B_WHH0N + NB]),
                (mc[0:18, C_W0TRZ:C_W0TRZ + RZ], mc[0:18, C_H0T:C_H0T + 2],
                 None, mc[0:18, C_W0TN:C_W0TN + NB]),
            ]
            g0_rz = pp.tile([C, 8], F32, tag="p1")
            g0_ni = pp.tile([C, 4], F32, tag="p2")
            g0_nh = pp.tile([C, 4], F32, tag="p3")
            wih0_mains, wih0_tail = wih0_chunks[0:4], wih0_chunks[4:5]
            whh0_main, whh0_tail = whh0_chunks[0:1], whh0_chunks[1:2]
            n0 = 4 * (len(wih0_chunks) + len(whh0_chunks))
            # same-PE-row-region chunks adjacent (mains 0:128 together,
            # then the 0:49 / 64:82 tails) to minimize stationary-region
            # switch drains
            with tc.high_priority(offset=40):
                i = _gru_mm_rz(nc, g0_rz, wih0_mains, 0, n0)
                i = _gru_mm_rz(nc, g0_rz, whh0_main, i, n0)
                i = _gru_mm_rz(nc, g0_rz, wih0_tail, i, n0)
                _gru_mm_rz(nc, g0_rz, whh0_tail, i, n0)
            _gru_mm_n(nc, g0_ni, wih0_mains, 0, 10)
            _gru_mm_n(nc, g0_nh, whh0_main, 0, 4)
            _gru_mm_n(nc, g0_ni, wih0_tail, 8, 10)
            _gru_mm_n(nc, g0_nh, whh0_tail, 2, 4)

            # --- GRU 1 h-dependent matmuls (overlap gru0 pointwise) ---
            # tail first: its bytes (mc) arrive long before whh1-main (b23),
            # and 0:18 follows gru0's last 0:18 region for free
            whh1_chunks = [
                (mc[0:18, C_W1TRZ:C_W1TRZ + RZ], mc[0:18, C_H1T:C_H1T + 2],
                 None, mc[0:18, C_W1TN:C_W1TN + NB]),
                (mb[0:128, B_WHH1:B_WHH1 + M3], mb[0:128, B_HK + 2:B_HK + 4],
                 None, mb[0:128, B_WHH1 + 2 * GP:B_WHH1 + 2 * GP + NB]),
            ]
            g1_rz = pp.tile([C, 8], F32, tag="p4")
            g1_ni = pp.tile([C, 4], F32, tag="p6")
            g1_nh = pp.tile([C, 4], F32, tag="p7")
            n1 = 4 * (len(whh1_chunks) + 2)
            with tc.high_priority(offset=30):
                i = _gru_mm_rz(nc, g1_rz, whh1_chunks, 0, n1)
            _gru_mm_n(nc, g1_nh, whh1_chunks, 0, 2 * len(whh1_chunks))

            # --- GRU 0 pointwise (Scalar/Vector; PE continues above) ---
            hp0 = _gru_pt(nc, ab, "g0", g0_rz, g0_ni, g0_nh, h0_sb)

            # --- GRU 1 input-dependent matmuls ---
            wih1_chunks = [
                (mb[0:C, B_WIH1 + c * M3: B_WIH1 + (c + 1) * M3],
                 hp0[:, 2 * c:2 * c + 2], None,
                 mb[0:C, B_WIH1 + c * M3 + 2 * GP: B_WIH1 + c * M3 + 2 * GP + NB])
                for c in range(2)
            ]
            with tc.high_priority(offset=30):
                _gru_mm_rz(nc, g1_rz, wih1_chunks, i, n1)
            _gru_mm_n(nc, g1_ni, wih1_chunks, 0, 2 * len(wih1_chunks))

            # --- GRU 1 pointwise ---
            hp1 = _gru_pt(nc, ab, "g1", g1_rz, g1_ni, g1_nh, h1_sb)

            # --- l2 ---
            ps_a = pp.tile([H2 + 1, 2], F32, tag="p3")
            for c in range(2):
                nc.tensor.matmul(
                    ps_a, mb[0:C, B_W2A + c * (H2 + 1): B_W2A + (c + 1) * (H2 + 1)],
                    hp1[:, 2 * c:2 * c + 2], start=(c == 0), stop=(c == 1),
                    skip_group_check=True)
            l2h = ab.tile([H2 + 1, 2], F16, tag="l2h")
            nc.vector.tensor_scalar_max(l2h, ps_a, 0.0)
            ps_o = pp.tile([1, NO], F32, tag="p5")
            nc.tensor.matmul(ps_o, l2h[:, 0:1],
                             mb[0:H2 + 1, B_W2B:B_W2B + NO],
                             start=True, stop=True, skip_group_check=True)
            out_sb = ab.tile([1, NO], F32, tag="out_sb")
            nc.vector.tensor_copy(out_sb, ps_o)
            nc.gpsimd.dma_start(d_out, out_sb, single_packet=True)

    nc.compile()
    return nc


def _get_nc():
    global _BUILT
    if _BUILT is None:
        _BUILT = _build()
    return _BUILT


def _gate_pack(W, b, z_pad_bias=0.0):
    """W:(435,K), b:(435,) -> (K+1, 438): W.T + bias row, per-gate 146-col
    blocks (zero pad col). z_pad_bias=100 on the ih matrix makes the h'
    garbage slot compute to exactly 1.0."""
    K = W.shape[1]
    full = np.concatenate([W.T, b[None, :]], axis=0).astype(np.float32)
    out = np.zeros((K + 1, M3), np.float32)
    for g in range(3):
        out[:, g * GP: g * GP + G] = full[:, g * G: (g + 1) * G]
    out[K, GP + G] = z_pad_bias
    return out


def pack_inputs(inputs):
    f = lambda a: np.asarray(a, np.float32)
    wih0 = _gate_pack(f(inputs["Wih0"]), f(inputs["bih0"]), 100.0)  # (561, 438)
    ma = np.zeros((128, A_F), np.float32)
    for c in range(4):
        rows = wih0[c * 128:(c + 1) * 128, :]
        ma[:, c * RZ:(c + 1) * RZ] = rows[:, 0:RZ]
        ma[:, A_RZ + c * NB:A_RZ + (c + 1) * NB] = rows[:, RZ:M3]

    mb = np.zeros((128, B_F), np.float32)
    mc = np.zeros((18, C_F2), np.float32)
    hn = f(inputs["hn"])
    for i, (col, h) in enumerate(((B_H0, hn[0]), (B_H1, hn[1]))):
        hx = np.append(h, np.float32(1.0))                 # (146,)
        v = hx.reshape(2, C).T                             # [73,2]
        mb[0:C, col:col + 4] = v[:, [0, 0, 1, 1]]          # ptwise pairs
        mb[0:128, B_HK + 2 * i:B_HK + 2 * i + 2] = hx[0:128, None]  # K-chunk rhs
        ct = C_H0T if i == 0 else C_H1T
        mc[0:18, ct:ct + 2] = hx[128:146, None]            # tail rhs (mc rows 0:18)
    mb[0:49, B_TRZ:B_TRZ + RZ] = wih0[512:561, 0:RZ]
    mb[0:49, B_TN:B_TN + NB] = wih0[512:561, RZ:M3]
    whh0 = _gate_pack(f(inputs["Whh0"]), f(inputs["bhh0"]))
    mb[0:128, B_WHH0RZ:B_WHH0RZ + RZ] = whh0[0:128, 0:RZ]
    mb[0:128, B_WHH0N:B_WHH0N + NB] = whh0[0:128, RZ:M3]
    mc[0:18, C_W0TRZ:C_W0TRZ + RZ] = whh0[128:146, 0:RZ]
    mc[0:18, C_W0TN:C_W0TN + NB] = whh0[128:146, RZ:M3]
    whh1 = _gate_pack(f(inputs["Whh1"]), f(inputs["bhh1"]))
    mb[0:128, B_WHH1:B_WHH1 + M3] = whh1[0:128, :]
    mc[0:18, C_W1TRZ:C_W1TRZ + RZ] = whh1[128:146, 0:RZ]
    mc[0:18, C_W1TN:C_W1TN + NB] = whh1[128:146, RZ:M3]
    wih1 = _gate_pack(f(inputs["Wih1"]), f(inputs["bih1"]), 100.0)
    mb[0:C, B_WIH1:B_WIH1 + M3] = wih1[0:C, :]
    mb[0:C, B_WIH1 + M3:B_WIH1 + 2 * M3] = wih1[C:2 * C, :]
    w2a = np.zeros((2 * C, H2 + 1), np.float32)
    w2a[0:G + 1, 0:H2] = np.concatenate(
        [f(inputs["W2a"]).T, f(inputs["b2a"])[None, :]], axis=0)
    w2a[G, H2] = 1.0                 # unit col -> l2h slot computes to 1.0
    mb[0:C, B_W2A:B_W2A + H2 + 1] = w2a[0:C, :]
    mb[0:C, B_W2A + H2 + 1:B_W2A + 2 * (H2 + 1)] = w2a[C:2 * C, :]
    w2b = np.concatenate([f(inputs["W2b"]).T, f(inputs["b2b"])[None, :]], axis=0)
    mb[0:H2 + 1, B_W2B:B_W2B + NO] = w2b

    mc[0:15, 0:H1] = np.concatenate(
        [f(inputs["W1"]).T, f(inputs["b1"])[None, :]], axis=0)
    mc[14, H1] = 1.0                 # unit col -> l1 slot computes to 1.0
    x_ext = np.concatenate([
        f(inputs["state_inno"]), f(inputs["obs_inno"]),
        f(inputs["diff_state"]), f(inputs["diff_obs"]), [np.float32(1.0)],
    ])
    mc[0:15, H1 + 1] = x_ext
    mc[0:15, H1 + 2] = x_ext
    h = lambda a: np.ascontiguousarray(a.astype(np.float16))
    return {"mega_a1": h(ma[:, 0:A_RZ]),
            "mega_a2": h(ma[:, A_RZ:A_F]),
            "mega_b1": h(mb[:, 0:B_WHH0N]),
            "mega_b1n": h(mb[:, B_WHH0N:B_MB1]),
            "mega_b23": h(mb[:, B_MB1:B_F]),
            "mega_c": h(mc)}


def kernel(**inputs):
    nc = _get_nc()
    in_map = pack_inputs(inputs)
    res = bass_utils.run_bass_kernel_spmd(nc, [in_map], core_ids=[0], trace=TRACE)
    kernel.last_result = res
    return np.asarray(res.results[0]["out"], np.float32).reshape(X_DIM, Y_DIM)


# revision 31
# speedup vs baseline: 1.0049x; 1.0049x over previous
"""KalmanNet SLAM DNN forward pass on a single Trainium2 NeuronCore.

Network: x(14) -> Linear(560)+ReLU -> GRUCell(145) -> GRUCell(145)
         -> Linear(40)+ReLU -> Linear(10) -> reshape (5,2)

~1.8MB of fp32 weights, single sample => memory-bound; replicate on one
core (per sharding hint).

v4: weights/activations in fp16 (halves HBM traffic; PE runs 16-bit
matmuls at 1 cycle/row at any pstate, vs fp32r's 4x penalty), psum fp32.
mc goes out on the Sync HWDGE ring (desc-gen concurrent with GpSimd's);
the weight images stream through the gpsimd SWDGE queue in compute
order (wih0-half1 -> whh0+tails -> wih0-half2 -> whh1 -> rest), each
with its own completion semaphore so consumers unblock as their slice
lands.  GRU1's h-dependent matmuls (whh1 @ h1) are emitted before
GRU0's pointwise chain so the PE keeps working through the
Scalar/Vector hops; rz matmuls get scheduler priority so the sigmoid
fires as early as possible, and the gate math is arranged as
h' = (1-z)*n + z*h with u=1-z and z*h computed while tanh runs.

Matvecs run weights-stationary on the TensorEngine.  Activation vectors
kept in duplicated column pairs ([K,2] rhs -> [M,2] psum) end to end.

Host-side numpy packs everything into partition-major DRAM images,
weights pre-transposed to [K, M] layout, biases folded as an extra
weight row against a constant-1.0 input element, GRU gates padded
145->146 so output chunks are uniform 73 partitions, and the z-gate
pad-column bias set to 100 so the h' garbage slot computes to exactly
the 1.0 the next bias row needs.
"""

import numpy as np

import concourse.bacc as bacc
import concourse.mybir as mybir
import concourse.tile as tile
from concourse import bass_utils
from concourse.alu_op_type import AluOpType as ALU

F32 = mybir.dt.float32
F16 = mybir.dt.float16
AF = mybir.ActivationFunctionType

X_DIM, Y_DIM = 5, 2
H1, H2 = 560, 40
G = 145          # GRU hidden size
C = 73           # partition chunk for the GRU state (2*73 = 146 = G+1)
GP = 2 * C       # per-gate padded column block
M3 = 3 * GP      # 438 padded gate columns
NO = X_DIM * Y_DIM

RZ = 2 * GP      # 292 rz columns per chunk (gru0 split layout)
NB = GP          # 146 n columns per chunk

# megaB (128-partition image) column map
B_H0, B_H1 = 0, 4                      # h ptwise pairs [73,4]: (c0,c0,c1,c1)
B_HK = 8                               # whh rhs chunks: h0[128], h1[128],
                                       # tails (h0 rows 64:82, h1 rows 96:114)
B_WHH0RZ = 14                          # [128, 292] whh0 rz cols
B_TRZ = B_WHH0RZ + RZ                  # [*, 292] tails rz: rows 0:49 wih0c4,
                                       #   64:82 whh0t, 96:114 whh1t
B_WHH0N = B_TRZ + RZ                   # [128, 146] whh0 n cols
B_TN = B_WHH0N + NB                    # [*, 146] tails n cols
B_MB1 = B_TN + NB                      # 890: end of DMA chunk 1 (rz part ends B_WHH0N)
B_WHH1 = B_MB1                         # [128, 438] K-chunk 0 (DMA chunk 2)
B_MB2 = B_WHH1 + M3                    # 1328: end of DMA chunk 2
B_WIH1 = B_MB2                         # 2 x 438 (73-row chunks, DMA chunk 3)
B_W2A = B_WIH1 + 2 * M3                # 2 x 41 (41st col makes the 1.0)
B_W2B = B_W2A + 2 * (H2 + 1)           # [41,10]
B_F = B_W2B + NO                       # 2296

A_F = 4 * M3                           # mega128: wih0 rz c0..c3 then n c0..c3
A_RZ = 4 * RZ                          # 1168: end of wih0 rz block
C_F = H1 + 3                           # megaC: W1T(561, unit col) + x pair
# whh tails live in the mc image at rows 0:18 (PE-row subset of the mains)
C_W0TRZ = C_F                          # [18, 292] whh0 tail rz
C_W0TN = C_W0TRZ + RZ                  # [18, 146] whh0 tail n
C_W1TRZ = C_W0TN + NB                  # [18, 292] whh1 tail rz
C_W1TN = C_W1TRZ + RZ                  # [18, 146] whh1 tail n
C_H0T = C_W1TN + NB                    # [18, 2] h0 tail rhs pair
C_H1T = C_H0T + 2                      # [18, 2] h1 tail rhs pair
C_F2 = C_H1T + 2                       # 1447 total mc cols

TRACE = False
_BUILT = None


def _tp(chunk):
    return chunk[2] if len(chunk) > 2 else None


def _gru_mm_rz(nc, ps_rz, chunks, i0, n_total):
    """Emit rz matmuls for `chunks` into ps_rz (accumulation i0..)."""
    i = i0
    for ch in chunks:
        lhsT, rhs = ch[0], ch[1]
        for g in (0, 1):
            for c in (0, 1):
                j = 2 * g + c
                nc.tensor.matmul(
                    ps_rz[:, 2 * j: 2 * j + 2],
                    lhsT[:, g * GP + C * c: g * GP + C * (c + 1)],
                    rhs, start=(i == 0), stop=(i == n_total - 1),
                    tile_position=_tp(ch), skip_group_check=True)
                i += 1
    return i


def _gru_mm_n(nc, ps, chunks, ni0, nn_total):
    """Emit n-gate matmuls for `chunks` into ps.  Each chunk is
    (rz_lhsT, rhs, tile_position|None, n_lhsT); the n block is sliced from
    the chunk's own n_lhsT base."""
    n_i = ni0
    for ch in chunks:
        n_lhsT, rhs = ch[3], ch[1]
        for c in (0, 1):
            nc.tensor.matmul(
                ps[:, 2 * c: 2 * c + 2],
                n_lhsT[:, C * c: C * (c + 1)],
                rhs, start=(n_i == 0), stop=(n_i == nn_total - 1),
                tile_position=_tp(ch), skip_group_check=True)
            n_i += 1
    return n_i


def _gru_pt(nc, ab, name, ps_rz, ps_ni, ps_nh, h_sb):
    """r,z = sigmoid(rz sums); n = tanh(i_n + r*h_n);
    h' = (1-z)*n + z*h.  u=1-z and zh=z*h are computed while tanh runs so
    only two dependent Vector ops remain after it.
    Returns h' [73,4] F16 pairs."""
    rz = ab.tile([C, 8], F32, tag=f"{name}_rz")
    nc.scalar.activation(rz[:, 0:4], ps_rz[:, 0:4], AF.Sigmoid)
    nc.scalar.activation(rz[:, 4:8], ps_rz[:, 4:8], AF.Sigmoid)
    t1 = ab.tile([C, 4], F32, tag=f"{name}_t1")
    nc.vector.tensor_mul(t1, rz[:, 0:4], ps_nh)
    nc.vector.tensor_add(t1, t1, ps_ni)
    n_sb = ab.tile([C, 4], F32, tag=f"{name}_n")
    nc.scalar.activation(n_sb, t1, AF.Tanh)
    u = ab.tile([C, 4], F32, tag=f"{name}_u")
    nc.vector.tensor_scalar(u, rz[:, 4:8], -1.0, 1.0, ALU.mult, ALU.add)
    zh = ab.tile([C, 4], F32, tag=f"{name}_zh")
    nc.vector.tensor_mul(zh, rz[:, 4:8], h_sb)
    t2 = ab.tile([C, 4], F32, tag=f"{name}_t2")
    nc.vector.tensor_mul(t2, n_sb, u)
    hp = ab.tile([C, 4], F16, tag=f"{name}_hp")
    nc.vector.tensor_add(hp, t2, zh)
    return hp


def _build():
    nc = bacc.Bacc("TRN2", num_devices=1, num_swdge_queues=4)

    d_c = nc.dram_tensor("mega_c", [18, C_F2], F16, kind="ExternalInput").ap()
    d_a1 = nc.dram_tensor("mega_a1", [128, A_RZ], F16, kind="ExternalInput").ap()
    d_a2 = nc.dram_tensor("mega_a2", [128, A_F - A_RZ], F16, kind="ExternalInput").ap()
    d_b1 = nc.dram_tensor("mega_b1", [128, B_WHH0N], F16, kind="ExternalInput").ap()
    d_b1n = nc.dram_tensor("mega_b1n", [128, B_MB1 - B_WHH0N], F16,
                           kind="ExternalInput").ap()
    d_b23 = nc.dram_tensor("mega_b23", [128, B_F - B_MB1], F16,
                           kind="ExternalInput").ap()
    d_out = nc.dram_tensor("out", [1, NO], F32, kind="ExternalOutput").ap()

    with tile.TileContext(nc) as tc:
        with (
            tc.tile_pool(name="wp", bufs=1) as wp,
            tc.tile_pool(name="ab", bufs=1) as ab,
            tc.tile_pool(name="pp", bufs=1, space="PSUM") as pp,
        ):
            # --- DMAs first: mc on the Sync HWDGE ring (concurrent
            # desc-gen), weights on SWDGE q0 in compute order ---
            mc = wp.tile([18, C_F2], F16, tag="mc")
            nc.sync.dma_start(mc, d_c)
            ma = wp.tile([128, A_F], F16, tag="ma")
            mb = wp.tile([128, B_F], F16, tag="mb")
            nc.gpsimd.dma_start(ma[:, 0:A_RZ], d_a1)
            nc.gpsimd.dma_start(mb[:, 0:B_WHH0N], d_b1)
            nc.gpsimd.dma_start(ma[:, A_RZ:A_F], d_a2)
            nc.gpsimd.dma_start(mb[:, B_WHH0N:B_MB1], d_b1n)
            nc.gpsimd.dma_start(mb[:, B_MB1:B_F], d_b23)

            # ACT table warmup (pulls the table-set load to t~0 on Scalar)
            warm = ab.tile([1, 1], F32, tag="warm")
            nc.vector.memset(warm, 0.0)
            warm2 = ab.tile([1, 1], F32, tag="warm2")
            nc.scalar.activation(warm2, warm, AF.Sigmoid)
            nc.scalar.activation(warm2, warm2, AF.Tanh)

            # h pairs to fp32 for the pointwise math (Vector idle then)
            h0_sb = ab.tile([C, 4], F32, tag="h0c")
            nc.vector.tensor_copy(h0_sb, mb[0:C, B_H0:B_H0 + 4])
            h1_sb = ab.tile([C, 4], F32, tag="h1c")
            nc.vector.tensor_copy(h1_sb, mb[0:C, B_H1:B_H1 + 4])

            # --- layer 1: l1 = relu(W1 @ x + b1), [128,10] paired cols;
            # relu split per chunk so gru0 chunk c can start asap ---
            x2 = mc[0:15, H1 + 1:H1 + 3]
            l1_sb = ab.tile([128, 10], F16, tag="l1")
            ps_l1 = pp.tile([128, 8], F32, tag="p0")
            for c in range(4):
                nc.tensor.matmul(ps_l1[:, 2 * c:2 * c + 2],
                                 mc[0:15, c * 128:(c + 1) * 128], x2,
                                 start=(c == 0), stop=(c == 3),
                                 skip_group_check=True)
            ps_l1b = pp.tile([49, 2], F32, tag="p5")
            nc.tensor.matmul(ps_l1b, mc[0:15, 512:561], x2,
                             start=True, stop=True)
            nc.vector.tensor_scalar_max(l1_sb[:, 0:8], ps_l1, 0.0)
            nc.vector.tensor_scalar_max(l1_sb[0:49, 8:10], ps_l1b, 0.0)

            # --- GRU 0 matmuls (split rz/n image layout) ---
            wih0_chunks = [
                (ma[:, c * RZ:(c + 1) * RZ], l1_sb[:, 2 * c:2 * c + 2], None,
                 ma[:, A_RZ + c * NB:A_RZ + (c + 1) * NB])
                for c in range(4)
            ] + [
                (mb[0:49, B_TRZ:B_TRZ + RZ], l1_sb[0:49, 8:10], None,
                 mb[0:49, B_TN:B_TN + NB])
            ]
            whh0_chunks = [
                (mb[0:128, B_WHH0RZ:B_WHH0RZ + RZ], mb[0:128, B_HK:B_HK + 2],
                 None, mb[0:128, B_WHH0N:B_WHH0N + NB]),
                (mc[0:18, C_W0TRZ:C_W0TRZ + RZ], mc[0:18, C_H0T:C_H0T + 2],
                 None, mc[0:18, C_W0TN:C_W0TN + NB]),
            ]
            g0_rz = pp.tile([C, 8], F32, tag="p1")
            g0_ni = pp.tile([C, 4], F32, tag="p2")
            g0_nh = pp.tile([C, 4], F32, tag="p3")
            wih0_mains, wih0_tail = wih0_chunks[0:4], wih0_chunks[4:5]
            whh0_main, whh0_tail = whh0_chunks[0:1], whh0_chunks[1:2]
            n0 = 4 * (len(wih0_chunks) + len(whh0_chunks))
            # same-PE-row-region chunks adjacent (mains 0:128 together,
            # then the 0:49 / 64:82 tails) to minimize stationary-region
            # switch drains
            with tc.high_priority(offset=40):
                i = _gru_mm_rz(nc, g0_rz, wih0_mains, 0, n0)
                i = _gru_mm_rz(nc, g0_rz, whh0_main, i, n0)
                i = _gru_mm_rz(nc, g0_rz, wih0_tail, i, n0)
                _gru_mm_rz(nc, g0_rz, whh0_tail, i, n0)
            _gru_mm_n(nc, g0_ni, wih0_mains, 0, 10)
            _gru_mm_n(nc, g0_nh, whh0_main, 0, 4)
            _gru_mm_n(nc, g0_ni, wih0_tail, 8, 10)
            _gru_mm_n(nc, g0_nh, whh0_tail, 2, 4)

            # --- GRU 1 h-dependent matmuls (overlap gru0 pointwise) ---
            # tail first: its bytes (mc) arrive long before whh1-main (b23),
            # and 0:18 follows gru0's last 0:18 region for free
            whh1_chunks = [
                (mc[0:18, C_W1TRZ:C_W1TRZ + RZ], mc[0:18, C_H1T:C_H1T + 2],
                 None, mc[0:18, C_W1TN:C_W1TN + NB]),
                (mb[0:128, B_WHH1:B_WHH1 + M3], mb[0:128, B_HK + 2:B_HK + 4],
                 None, mb[0:128, B_WHH1 + 2 * GP:B_WHH1 + 2 * GP + NB]),
            ]
            g1_rz = pp.tile([C, 8], F32, tag="p4")
            g1_ni = pp.tile([C, 4], F32, tag="p6")
            g1_nh = pp.tile([C, 4], F32, tag="p7")
            n1 = 4 * (len(whh1_chunks) + 2)
            with tc.high_priority(offset=30):
                i = _gru_mm_rz(nc, g1_rz, whh1_chunks, 0, n1)
            _gru_mm_n(nc, g1_nh, whh1_chunks, 0, 2 * len(whh1_chunks))

            # --- GRU 0 pointwise (Scalar/Vector; PE continues above) ---
            hp0 = _gru_pt(nc, ab, "g0", g0_rz, g0_ni, g0_nh, h0_sb)

            # --- GRU 1 input-dependent matmuls ---
            wih1_chunks = [
                (mb[0:C, B_WIH1 + c * M3: B_WIH1 + (c + 1) * M3],
                 hp0[:, 2 * c:2 * c + 2], None,
                 mb[0:C, B_WIH1 + c * M3 + 2 * GP: B_WIH1 + c * M3 + 2 * GP + NB])
                for c in range(2)
            ]
            with tc.high_priority(offset=30):
                _gru_mm_rz(nc, g1_rz, wih1_chunks, i, n1)
            _gru_mm_n(nc, g1_ni, wih1_chunks, 0, 2 * len(wih1_chunks))

            # --- GRU 1 pointwise ---
            hp1 = _gru_pt(nc, ab, "g1", g1_rz, g1_ni, g1_nh, h1_sb)

            # --- l2 ---
            ps_a = pp.tile([H2 + 1, 2], F32, tag="p3")
            for c in range(2):
                nc.tensor.matmul(
                    ps_a, mb[0:C, B_W2A + c * (H2 + 1): B_W2A + (c + 1) * (H2 + 1)],
                    hp1[:, 2 * c:2 * c + 2], start=(c == 0), stop=(c == 1),
                    skip_group_check=True)
            l2h = ab.tile([H2 + 1, 2], F16, tag="l2h")
            nc.vector.tensor_scalar_max(l2h, ps_a, 0.0)
            ps_o = pp.tile([1, NO], F32, tag="p5")
            nc.tensor.matmul(ps_o, l2h[:, 0:1],
                             mb[0:H2 + 1, B_W2B:B_W2B + NO],
                             start=True, stop=True, skip_group_check=True)
            out_sb = ab.tile([1, NO], F32, tag="out_sb")
            nc.vector.tensor_copy(out_sb, ps_o)
            nc.gpsimd.dma_start(d_out, out_sb)

    nc.compile()
    return nc


def _get_nc():
    global _BUILT
    if _BUILT is None:
        _BUILT = _build()
    return _BUILT


def _gate_pack(W, b, z_pad_bias=0.0):
    """W:(435,K), b:(435,) -> (K+1, 438): W.T + bias row, per-gate 146-col
    blocks (zero pad col). z_pad_bias=100 on the ih matrix makes the h'
    garbage slot compute to exactly 1.0."""
    K = W.shape[1]
    full = np.concatenate([W.T, b[None, :]], axis=0).astype(np.float32)
    out = np.zeros((K + 1, M3), np.float32)
    for g in range(3):
        out[:, g * GP: g * GP + G] = full[:, g * G: (g + 1) * G]
    out[K, GP + G] = z_pad_bias
    return out


def pack_inputs(inputs):
    f = lambda a: np.asarray(a, np.float32)
    wih0 = _gate_pack(f(inputs["Wih0"]), f(inputs["bih0"]), 100.0)  # (561, 438)
    ma = np.zeros((128, A_F), np.float32)
    for c in range(4):
        rows = wih0[c * 128:(c + 1) * 128, :]
        ma[:, c * RZ:(c + 1) * RZ] = rows[:, 0:RZ]
        ma[:, A_RZ + c * NB:A_RZ + (c + 1) * NB] = rows[:, RZ:M3]

    mb = np.zeros((128, B_F), np.float32)
    mc = np.zeros((18, C_F2), np.float32)
    hn = f(inputs["hn"])
    for i, (col, h) in enumerate(((B_H0, hn[0]), (B_H1, hn[1]))):
        hx = np.append(h, np.float32(1.0))                 # (146,)
        v = hx.reshape(2, C).T                             # [73,2]
        mb[0:C, col:col + 4] = v[:, [0, 0, 1, 1]]          # ptwise pairs
        mb[0:128, B_HK + 2 * i:B_HK + 2 * i + 2] = hx[0:128, None]  # K-chunk rhs
        ct = C_H0T if i == 0 else C_H1T
        mc[0:18, ct:ct + 2] = hx[128:146, None]            # tail rhs (mc rows 0:18)
    mb[0:49, B_TRZ:B_TRZ + RZ] = wih0[512:561, 0:RZ]
    mb[0:49, B_TN:B_TN + NB] = wih0[512:561, RZ:M3]
    whh0 = _gate_pack(f(inputs["Whh0"]), f(inputs["bhh0"]))
    mb[0:128, B_WHH0RZ:B_WHH0RZ + RZ] = whh0[0:128, 0:RZ]
    mb[0:128, B_WHH0N:B_WHH0N + NB] = whh0[0:128, RZ:M3]
    mc[0:18, C_W0TRZ:C_W0TRZ + RZ] = whh0[128:146, 0:RZ]
    mc[0:18, C_W0TN:C_W0TN + NB] = whh0[128:146, RZ:M3]
    whh1 = _gate_pack(f(inputs["Whh1"]), f(inputs["bhh1"]))
    mb[0:128, B_WHH1:B_WHH1 + M3] = whh1[0:128, :]
    mc[0:18, C_W1TRZ:C_W1TRZ + RZ] = whh1[128:146, 0:RZ]
    mc[0:18, C_W1TN:C_W1TN + NB] = whh1[128:146, RZ:M3]
    wih1 = _gate_pack(f(inputs["Wih1"]), f(inputs["bih1"]), 100.0)
    mb[0:C, B_WIH1:B_WIH1 + M3] = wih1[0:C, :]
    mb[0:C, B_WIH1 + M3:B_WIH1 + 2 * M3] = wih1[C:2 * C, :]
    w2a = np.zeros((2 * C, H2 + 1), np.float32)
    w2a[0:G + 1, 0:H2] = np.concatenate(
        [f(inputs["W2a"]).T, f(inputs["b2a"])[None, :]], axis=0)
    w2a[G, H2] = 1.0                 # unit col -> l2h slot computes to 1.0
    mb[0:C, B_W2A:B_W2A + H2 + 1] = w2a[0:C, :]
    mb[0:C, B_W2A + H2 + 1:B_W2A + 2 * (H2 + 1)] = w2a[C:2 * C, :]
    w2b = np.concatenate([f(inputs["W2b"]).T, f(inputs["b2b"])[None, :]], axis=0)
    mb[0:H2 + 1, B_W2B:B_W2B + NO] = w2b

    mc[0:15, 0:H1] = np.concatenate(
        [f(inputs["W1"]).T, f(inputs["b1"])[None, :]], axis=0)
    mc[14, H1] = 1.0                 # unit col -> l1 slot computes to 1.0
    x_ext = np.concatenate([
        f(inputs["state_inno"]), f(inputs["obs_inno"]),
        f(inputs["diff_state"]), f(inputs["diff_obs"]), [np.float32(1.0)],
    ])
    mc[0:15, H1 + 1] = x_ext
    mc[0:15, H1 + 2] = x_ext
    h = lambda a: np.ascontiguousarray(a.astype(np.float16))
    return {"mega_a1": h(ma[:, 0:A_RZ]),
            "mega_a2": h(ma[:, A_RZ:A_F]),
            "mega_b1": h(mb[:, 0:B_WHH0N]),
            "mega_b1n": h(mb[:, B_WHH0N:B_MB1]),
            "mega_b23": h(mb[:, B_MB1:B_F]),
            "mega_c": h(mc)}


def kernel(**inputs):
    nc = _get_nc()
    in_map = pack_inputs(inputs)
    res = bass_utils.run_bass_kernel_spmd(nc, [in_map], core_ids=[0], trace=TRACE)
    kernel.last_result = res
    return np.asarray(res.results[0]["out"], np.float32).reshape(X_DIM, Y_DIM)


# revision 32
# speedup vs baseline: 1.0236x; 1.0186x over previous
"""KalmanNet SLAM DNN forward pass on a single Trainium2 NeuronCore.

Network: x(14) -> Linear(560)+ReLU -> GRUCell(145) -> GRUCell(145)
         -> Linear(40)+ReLU -> Linear(10) -> reshape (5,2)

~1.8MB of fp32 weights, single sample => memory-bound; replicate on one
core (per sharding hint).

v4: weights/activations in fp16 (halves HBM traffic; PE runs 16-bit
matmuls at 1 cycle/row at any pstate, vs fp32r's 4x penalty), psum fp32.
mc goes out on the Sync HWDGE ring (desc-gen concurrent with GpSimd's);
the weight images stream through the gpsimd SWDGE queue in compute
order (wih0-half1 -> whh0+tails -> wih0-half2 -> whh1 -> rest), each
with its own completion semaphore so consumers unblock as their slice
lands.  GRU1's h-dependent matmuls (whh1 @ h1) are emitted before
GRU0's pointwise chain so the PE keeps working through the
Scalar/Vector hops; rz matmuls get scheduler priority so the sigmoid
fires as early as possible, and the gate math is arranged as
h' = (1-z)*n + z*h with u=1-z and z*h computed while tanh runs.

Matvecs run weights-stationary on the TensorEngine.  Activation vectors
kept in duplicated column pairs ([K,2] rhs -> [M,2] psum) end to end.

Host-side numpy packs everything into partition-major DRAM images,
weights pre-transposed to [K, M] layout, biases folded as an extra
weight row against a constant-1.0 input element, GRU gates padded
145->146 so output chunks are uniform 73 partitions, and the z-gate
pad-column bias set to 100 so the h' garbage slot computes to exactly
the 1.0 the next bias row needs.
"""

import numpy as np

import concourse.bacc as bacc
import concourse.mybir as mybir
import concourse.tile as tile
from concourse import bass_utils
from concourse.alu_op_type import AluOpType as ALU

F32 = mybir.dt.float32
F16 = mybir.dt.float16
AF = mybir.ActivationFunctionType

X_DIM, Y_DIM = 5, 2
H1, H2 = 560, 40
G = 145          # GRU hidden size
C = 73           # partition chunk for the GRU state (2*73 = 146 = G+1)
GP = 2 * C       # per-gate padded column block
M3 = 3 * GP      # 438 padded gate columns
NO = X_DIM * Y_DIM

RZ = 2 * GP      # 292 rz columns per chunk (gru0 split layout)
NB = GP          # 146 n columns per chunk

# megaB (128-partition image) column map
B_H0, B_H1 = 0, 4                      # h ptwise pairs [73,4]: (c0,c0,c1,c1)
B_HK = 8                               # whh rhs chunks: h0[128], h1[128],
                                       # tails (h0 rows 64:82, h1 rows 96:114)
B_WHH0RZ = 14                          # [128, 292] whh0 rz cols
B_TRZ = B_WHH0RZ + RZ                  # [*, 292] tails rz: rows 0:49 wih0c4,
                                       #   64:82 whh0t, 96:114 whh1t
B_WHH0N = B_TRZ + RZ                   # [128, 146] whh0 n cols
B_TN = B_WHH0N + NB                    # [*, 146] tails n cols
B_MB1 = B_TN + NB                      # 890: end of DMA chunk 1 (rz part ends B_WHH0N)
B_WHH1 = B_MB1                         # [128, 438] K-chunk 0 (DMA chunk 2)
B_MB2 = B_WHH1 + M3                    # 1328: end of DMA chunk 2
B_WIH1 = B_MB2                         # 2 x 438 (73-row chunks, DMA chunk 3)
B_W2A = B_WIH1 + 2 * M3                # 2 x 41 (41st col makes the 1.0)
B_W2B = B_W2A + 2 * (H2 + 1)           # [41,10]
B_F = B_W2B + NO                       # 2296

A_F = 4 * M3                           # mega128: wih0 rz c0..c3 then n c0..c3
A_RZ = 4 * RZ                          # 1168: end of wih0 rz block
C_F = H1 + 3                           # megaC: W1T(561, unit col) + x pair
# whh tails live in the mc image at rows 0:18 (PE-row subset of the mains)
C_W0TRZ = C_F                          # [18, 292] whh0 tail rz
C_W0TN = C_W0TRZ + RZ                  # [18, 146] whh0 tail n
C_W1TRZ = C_W0TN + NB                  # [18, 292] whh1 tail rz
C_W1TN = C_W1TRZ + RZ                  # [18, 146] whh1 tail n
C_H0T = C_W1TN + NB                    # [18, 2] h0 tail rhs pair
C_H1T = C_H0T + 2                      # [18, 2] h1 tail rhs pair
C_F2 = C_H1T + 2                       # 1447 total mc cols

TRACE = False
_BUILT = None


def _tp(chunk):
    return chunk[2] if len(chunk) > 2 else None


def _gru_mm_rz(nc, ps_rz, chunks, i0, n_total):
    """Emit rz matmuls for `chunks` into ps_rz (accumulation i0..)."""
    i = i0
    for ch in chunks:
        lhsT, rhs = ch[0], ch[1]
        for g in (0, 1):
            for c in (0, 1):
                j = 2 * g + c
                nc.tensor.matmul(
                    ps_rz[:, 2 * j: 2 * j + 2],
                    lhsT[:, g * GP + C * c: g * GP + C * (c + 1)],
                    rhs, start=(i == 0), stop=(i == n_total - 1),
                    tile_position=_tp(ch), skip_group_check=True)
                i += 1
    return i


def _gru_mm_n(nc, ps, chunks, ni0, nn_total):
    """Emit n-gate matmuls for `chunks` into ps.  Each chunk is
    (rz_lhsT, rhs, tile_position|None, n_lhsT); the n block is sliced from
    the chunk's own n_lhsT base."""
    n_i = ni0
    for ch in chunks:
        n_lhsT, rhs = ch[3], ch[1]
        for c in (0, 1):
            nc.tensor.matmul(
                ps[:, 2 * c: 2 * c + 2],
                n_lhsT[:, C * c: C * (c + 1)],
                rhs, start=(n_i == 0), stop=(n_i == nn_total - 1),
                tile_position=_tp(ch), skip_group_check=True)
            n_i += 1
    return n_i


def _gru_pt(nc, ab, name, ps_rz, ps_ni, ps_nh, h_sb, terminal=False):
    """r,z = sigmoid(rz sums); n = tanh(i_n + r*h_n);
    h' = (1-z)*n + z*h.  u=1-z and zh=z*h are computed while tanh runs so
    only two dependent Vector ops remain after it.
    Returns h' [73,4] F16 pairs — unless terminal=True: then the final add
    is skipped and (t2, zh) F16 pairs are returned for the consumer to
    accumulate via matmul linearity (W @ (t2+zh) = W@t2 + W@zh in PSUM);
    zh is ready before tanh even finishes."""
    rz = ab.tile([C, 8], F32, tag=f"{name}_rz")
    nc.scalar.activation(rz[:, 0:4], ps_rz[:, 0:4], AF.Sigmoid)
    nc.scalar.activation(rz[:, 4:8], ps_rz[:, 4:8], AF.Sigmoid)
    t1 = ab.tile([C, 4], F32, tag=f"{name}_t1")
    nc.vector.tensor_mul(t1, rz[:, 0:4], ps_nh)
    nc.vector.tensor_add(t1, t1, ps_ni)
    n_sb = ab.tile([C, 4], F32, tag=f"{name}_n")
    nc.scalar.activation(n_sb, t1, AF.Tanh)
    u = ab.tile([C, 4], F32, tag=f"{name}_u")
    nc.vector.tensor_scalar(u, rz[:, 4:8], -1.0, 1.0, ALU.mult, ALU.add)
    zht = F16 if terminal else F32
    zh = ab.tile([C, 4], zht, tag=f"{name}_zh")
    nc.vector.tensor_mul(zh, rz[:, 4:8], h_sb)
    t2 = ab.tile([C, 4], zht, tag=f"{name}_t2")
    nc.vector.tensor_mul(t2, n_sb, u)
    if terminal:
        return t2, zh
    hp = ab.tile([C, 4], F16, tag=f"{name}_hp")
    nc.vector.tensor_add(hp, t2, zh)
    return hp


def _build():
    nc = bacc.Bacc("TRN2", num_devices=1, num_swdge_queues=4)

    d_c = nc.dram_tensor("mega_c", [18, C_F2], F16, kind="ExternalInput").ap()
    d_a1 = nc.dram_tensor("mega_a1", [128, A_RZ], F16, kind="ExternalInput").ap()
    d_a2 = nc.dram_tensor("mega_a2", [128, A_F - A_RZ], F16, kind="ExternalInput").ap()
    d_b1 = nc.dram_tensor("mega_b1", [128, B_WHH0N], F16, kind="ExternalInput").ap()
    d_b1n = nc.dram_tensor("mega_b1n", [128, B_MB1 - B_WHH0N], F16,
                           kind="ExternalInput").ap()
    d_b23 = nc.dram_tensor("mega_b23", [128, B_F - B_MB1], F16,
                           kind="ExternalInput").ap()
    d_out = nc.dram_tensor("out", [1, NO], F32, kind="ExternalOutput").ap()

    with tile.TileContext(nc) as tc:
        with (
            tc.tile_pool(name="wp", bufs=1) as wp,
            tc.tile_pool(name="ab", bufs=1) as ab,
            tc.tile_pool(name="pp", bufs=1, space="PSUM") as pp,
        ):
            # --- DMAs first: mc on the Sync HWDGE ring (concurrent
            # desc-gen), weights on SWDGE q0 in compute order ---
            mc = wp.tile([18, C_F2], F16, tag="mc")
            nc.sync.dma_start(mc, d_c)
            ma = wp.tile([128, A_F], F16, tag="ma")
            mb = wp.tile([128, B_F], F16, tag="mb")
            nc.gpsimd.dma_start(ma[:, 0:A_RZ], d_a1)
            nc.gpsimd.dma_start(mb[:, 0:B_WHH0N], d_b1)
            nc.gpsimd.dma_start(ma[:, A_RZ:A_F], d_a2)
            nc.gpsimd.dma_start(mb[:, B_WHH0N:B_MB1], d_b1n)
            nc.gpsimd.dma_start(mb[:, B_MB1:B_F], d_b23)

            # ACT table warmup (pulls the table-set load to t~0 on Scalar)
            warm = ab.tile([1, 1], F32, tag="warm")
            nc.vector.memset(warm, 0.0)
            warm2 = ab.tile([1, 1], F32, tag="warm2")
            nc.scalar.activation(warm2, warm, AF.Sigmoid)
            nc.scalar.activation(warm2, warm2, AF.Tanh)

            # h pairs to fp32 for the pointwise math (Vector idle then)
            h0_sb = ab.tile([C, 4], F32, tag="h0c")
            nc.vector.tensor_copy(h0_sb, mb[0:C, B_H0:B_H0 + 4])
            h1_sb = ab.tile([C, 4], F32, tag="h1c")
            nc.vector.tensor_copy(h1_sb, mb[0:C, B_H1:B_H1 + 4])

            # --- layer 1: l1 = relu(W1 @ x + b1), [128,10] paired cols;
            # relu split per chunk so gru0 chunk c can start asap ---
            x2 = mc[0:15, H1 + 1:H1 + 3]
            l1_sb = ab.tile([128, 10], F16, tag="l1")
            ps_l1 = pp.tile([128, 8], F32, tag="p0")
            for c in range(4):
                nc.tensor.matmul(ps_l1[:, 2 * c:2 * c + 2],
                                 mc[0:15, c * 128:(c + 1) * 128], x2,
                                 start=(c == 0), stop=(c == 3),
                                 skip_group_check=True)
            ps_l1b = pp.tile([49, 2], F32, tag="p5")
            nc.tensor.matmul(ps_l1b, mc[0:15, 512:561], x2,
                             start=True, stop=True)
            nc.vector.tensor_scalar_max(l1_sb[:, 0:8], ps_l1, 0.0)
            nc.vector.tensor_scalar_max(l1_sb[0:49, 8:10], ps_l1b, 0.0)

            # --- GRU 0 matmuls (split rz/n image layout) ---
            wih0_chunks = [
                (ma[:, c * RZ:(c + 1) * RZ], l1_sb[:, 2 * c:2 * c + 2], None,
                 ma[:, A_RZ + c * NB:A_RZ + (c + 1) * NB])
                for c in range(4)
            ] + [
                (mb[0:49, B_TRZ:B_TRZ + RZ], l1_sb[0:49, 8:10], None,
                 mb[0:49, B_TN:B_TN + NB])
            ]
            whh0_chunks = [
                (mb[0:128, B_WHH0RZ:B_WHH0RZ + RZ], mb[0:128, B_HK:B_HK + 2],
                 None, mb[0:128, B_WHH0N:B_WHH0N + NB]),
                (mc[0:18, C_W0TRZ:C_W0TRZ + RZ], mc[0:18, C_H0T:C_H0T + 2],
                 None, mc[0:18, C_W0TN:C_W0TN + NB]),
            ]
            g0_rz = pp.tile([C, 8], F32, tag="p1")
            g0_ni = pp.tile([C, 4], F32, tag="p2")
            g0_nh = pp.tile([C, 4], F32, tag="p3")
            wih0_mains, wih0_tail = wih0_chunks[0:4], wih0_chunks[4:5]
            whh0_main, whh0_tail = whh0_chunks[0:1], whh0_chunks[1:2]
            n0 = 4 * (len(wih0_chunks) + len(whh0_chunks))
            # same-PE-row-region chunks adjacent (mains 0:128 together,
            # then the 0:49 / 64:82 tails) to minimize stationary-region
            # switch drains
            with tc.high_priority(offset=40):
                i = _gru_mm_rz(nc, g0_rz, wih0_mains, 0, n0)
                i = _gru_mm_rz(nc, g0_rz, whh0_main, i, n0)
                i = _gru_mm_rz(nc, g0_rz, wih0_tail, i, n0)
                _gru_mm_rz(nc, g0_rz, whh0_tail, i, n0)
            _gru_mm_n(nc, g0_ni, wih0_mains, 0, 10)
            _gru_mm_n(nc, g0_nh, whh0_main, 0, 4)
            _gru_mm_n(nc, g0_ni, wih0_tail, 8, 10)
            _gru_mm_n(nc, g0_nh, whh0_tail, 2, 4)

            # --- GRU 1 h-dependent matmuls (overlap gru0 pointwise) ---
            # tail first: its bytes (mc) arrive long before whh1-main (b23),
            # and 0:18 follows gru0's last 0:18 region for free
            whh1_chunks = [
                (mc[0:18, C_W1TRZ:C_W1TRZ + RZ], mc[0:18, C_H1T:C_H1T + 2],
                 None, mc[0:18, C_W1TN:C_W1TN + NB]),
                (mb[0:128, B_WHH1:B_WHH1 + M3], mb[0:128, B_HK + 2:B_HK + 4],
                 None, mb[0:128, B_WHH1 + 2 * GP:B_WHH1 + 2 * GP + NB]),
            ]
            g1_rz = pp.tile([C, 8], F32, tag="p4")
            g1_ni = pp.tile([C, 4], F32, tag="p6")
            g1_nh = pp.tile([C, 4], F32, tag="p7")
            n1 = 4 * (len(whh1_chunks) + 2)
            with tc.high_priority(offset=30):
                i = _gru_mm_rz(nc, g1_rz, whh1_chunks, 0, n1)
            _gru_mm_n(nc, g1_nh, whh1_chunks, 0, 2 * len(whh1_chunks))

            # --- GRU 0 pointwise (Scalar/Vector; PE continues above) ---
            hp0 = _gru_pt(nc, ab, "g0", g0_rz, g0_ni, g0_nh, h0_sb)

            # --- GRU 1 input-dependent matmuls ---
            wih1_chunks = [
                (mb[0:C, B_WIH1 + c * M3: B_WIH1 + (c + 1) * M3],
                 hp0[:, 2 * c:2 * c + 2], None,
                 mb[0:C, B_WIH1 + c * M3 + 2 * GP: B_WIH1 + c * M3 + 2 * GP + NB])
                for c in range(2)
            ]
            with tc.high_priority(offset=30):
                _gru_mm_rz(nc, g1_rz, wih1_chunks, i, n1)
            _gru_mm_n(nc, g1_ni, wih1_chunks, 0, 2 * len(wih1_chunks))

            # --- GRU 1 pointwise (terminal: no hp1 add; l2 accumulates
            # W2a@zh + W2a@t2 in PSUM) ---
            t2_1, zh_1 = _gru_pt(nc, ab, "g1", g1_rz, g1_ni, g1_nh, h1_sb,
                                 terminal=True)

            # --- l2 ---
            ps_a = pp.tile([H2 + 1, 2], F32, tag="p3")
            for c in range(2):
                nc.tensor.matmul(
                    ps_a, mb[0:C, B_W2A + c * (H2 + 1): B_W2A + (c + 1) * (H2 + 1)],
                    zh_1[:, 2 * c:2 * c + 2], start=(c == 0), stop=False,
                    skip_group_check=True)
            for c in range(2):
                nc.tensor.matmul(
                    ps_a, mb[0:C, B_W2A + c * (H2 + 1): B_W2A + (c + 1) * (H2 + 1)],
                    t2_1[:, 2 * c:2 * c + 2], start=False, stop=(c == 1),
                    skip_group_check=True)
            l2h = ab.tile([H2 + 1, 2], F16, tag="l2h")
            nc.vector.tensor_scalar_max(l2h, ps_a, 0.0)
            ps_o = pp.tile([1, NO], F32, tag="p5")
            nc.tensor.matmul(ps_o, l2h[:, 0:1],
                             mb[0:H2 + 1, B_W2B:B_W2B + NO],
                             start=True, stop=True, skip_group_check=True)
            out_sb = ab.tile([1, NO], F32, tag="out_sb")
            nc.vector.tensor_copy(out_sb, ps_o)
            nc.gpsimd.dma_start(d_out, out_sb)

    nc.compile()
    return nc


def _get_nc():
    global _BUILT
    if _BUILT is None:
        _BUILT = _build()
    return _BUILT


def _gate_pack(W, b, z_pad_bias=0.0):
    """W:(435,K), b:(435,) -> (K+1, 438): W.T + bias row, per-gate 146-col
    blocks (zero pad col). z_pad_bias=100 on the ih matrix makes the h'
    garbage slot compute to exactly 1.0."""
    K = W.shape[1]
    full = np.concatenate([W.T, b[None, :]], axis=0).astype(np.float32)
    out = np.zeros((K + 1, M3), np.float32)
    for g in range(3):
        out[:, g * GP: g * GP + G] = full[:, g * G: (g + 1) * G]
    out[K, GP + G] = z_pad_bias
    return out


def pack_inputs(inputs):
    f = lambda a: np.asarray(a, np.float32)
    wih0 = _gate_pack(f(inputs["Wih0"]), f(inputs["bih0"]), 100.0)  # (561, 438)
    ma = np.zeros((128, A_F), np.float32)
    for c in range(4):
        rows = wih0[c * 128:(c + 1) * 128, :]
        ma[:, c * RZ:(c + 1) * RZ] = rows[:, 0:RZ]
        ma[:, A_RZ + c * NB:A_RZ + (c + 1) * NB] = rows[:, RZ:M3]

    mb = np.zeros((128, B_F), np.float32)
    mc = np.zeros((18, C_F2), np.float32)
    hn = f(inputs["hn"])
    for i, (col, h) in enumerate(((B_H0, hn[0]), (B_H1, hn[1]))):
        hx = np.append(h, np.float32(1.0))                 # (146,)
        v = hx.reshape(2, C).T                             # [73,2]
        mb[0:C, col:col + 4] = v[:, [0, 0, 1, 1]]          # ptwise pairs
        mb[0:128, B_HK + 2 * i:B_HK + 2 * i + 2] = hx[0:128, None]  # K-chunk rhs
        ct = C_H0T if i == 0 else C_H1T
        mc[0:18, ct:ct + 2] = hx[128:146, None]            # tail rhs (mc rows 0:18)
    mb[0:49, B_TRZ:B_TRZ + RZ] = wih0[512:561, 0:RZ]
    mb[0:49, B_TN:B_TN + NB] = wih0[512:561, RZ:M3]
    whh0 = _gate_pack(f(inputs["Whh0"]), f(inputs["bhh0"]))
    mb[0:128, B_WHH0RZ:B_WHH0RZ + RZ] = whh0[0:128, 0:RZ]
    mb[0:128, B_WHH0N:B_WHH0N + NB] = whh0[0:128, RZ:M3]
    mc[0:18, C_W0TRZ:C_W0TRZ + RZ] = whh0[128:146, 0:RZ]
    mc[0:18, C_W0TN:C_W0TN + NB] = whh0[128:146, RZ:M3]
    whh1 = _gate_pack(f(inputs["Whh1"]), f(inputs["bhh1"]))
    mb[0:128, B_WHH1:B_WHH1 + M3] = whh1[0:128, :]
    mc[0:18, C_W1TRZ:C_W1TRZ + RZ] = whh1[128:146, 0:RZ]
    mc[0:18, C_W1TN:C_W1TN + NB] = whh1[128:146, RZ:M3]
    wih1 = _gate_pack(f(inputs["Wih1"]), f(inputs["bih1"]), 100.0)
    mb[0:C, B_WIH1:B_WIH1 + M3] = wih1[0:C, :]
    mb[0:C, B_WIH1 + M3:B_WIH1 + 2 * M3] = wih1[C:2 * C, :]
    w2a = np.zeros((2 * C, H2 + 1), np.float32)
    w2a[0:G + 1, 0:H2] = np.concatenate(
        [f(inputs["W2a"]).T, f(inputs["b2a"])[None, :]], axis=0)
    w2a[G, H2] = 1.0                 # unit col -> l2h slot computes to 1.0
    mb[0:C, B_W2A:B_W2A + H2 + 1] = w2a[0:C, :]
    mb[0:C, B_W2A + H2 + 1:B_W2A + 2 * (H2 + 1)] = w2a[C:2 * C, :]
    w2b = np.concatenate([f(inputs["W2b"]).T, f(inputs["b2b"])[None, :]], axis=0)
    mb[0:H2 + 1, B_W2B:B_W2B + NO] = w2b

    mc[0:15, 0:H1] = np.concatenate(
        [f(inputs["W1"]).T, f(inputs["b1"])[None, :]], axis=0)
    mc[14, H1] = 1.0                 # unit col -> l1 slot computes to 1.0
    x_ext = np.concatenate([
        f(inputs["state_inno"]), f(inputs["obs_inno"]),
        f(inputs["diff_state"]), f(inputs["diff_obs"]), [np.float32(1.0)],
    ])
    mc[0:15, H1 + 1] = x_ext
    mc[0:15, H1 + 2] = x_ext
    h = lambda a: np.ascontiguousarray(a.astype(np.float16))
    return {"mega_a1": h(ma[:, 0:A_RZ]),
            "mega_a2": h(ma[:, A_RZ:A_F]),
            "mega_b1": h(mb[:, 0:B_WHH0N]),
            "mega_b1n": h(mb[:, B_WHH0N:B_MB1]),
            "mega_b23": h(mb[:, B_MB1:B_F]),
            "mega_c": h(mc)}


def kernel(**inputs):
    nc = _get_nc()
    in_map = pack_inputs(inputs)
    res = bass_utils.run_bass_kernel_spmd(nc, [in_map], core_ids=[0], trace=TRACE)
    kernel.last_result = res
    return np.asarray(res.results[0]["out"], np.float32).reshape(X_DIM, Y_DIM)
